# revision 48
# baseline (speedup 1.0000x reference)
"""Bass/Tile kernel for nn_CausalSelfAttention (GQA + RMS-norm + RoPE + sliding window).

Sharding: 4-way sequence x 2-way heads over 8 NeuronCores.
Per core: 1024 queries, 8 q-heads, 2 kv-heads, kv buffer of 2048 rows
(1024-row halo to the left, zero-padded for the first sequence shard).

All layouts are transpose-free on device:
  - host passes x^T and W^T slices
  - projections produce q^T/k^T [hd, seq] (lhsT = W tile) and v [seq, hd]
    (lhsT = x^T tile) directly
  - scores^T [sk, sq] = k_tile^T.T @ q^T ; PV: y^T += v_tile.T @ probs^T
  - out^T = Wo^T.T @ y^T  (partial over this core's heads; host sums pairs)

RMS-norm scales fold into the RoPE multiply; 1/sqrt(hd) folds into the
q-side scale; softmax needs no running max (rms-normed logits bounded by
sqrt(128)). Causal/window edges are handled by multiplying probs with
host-precomputed 0/1 mask tiles on DVE (cheap, keeps Pool free); the
sliding-window structure means only 4 of 6 sk-tile pairs per query block
need a mask. Padded halo keys project to v=0 so they never pollute y;
only the softmax denominator needs the pz zeroing on the first shard.

Attention runs on 256-query blocks (10 sk tiles each instead of 12 per
512 thanks to the sliding window), emitted as a 2-head depth-2 software
pipeline: each step issues the NEXT pair's score matmuls before the
current pair's PV matmuls, so the PE never stalls on the
Act-exp -> DVE-mask chain.  Wo stays resident in SBUF; rms-norm
partition reduction runs on the PE (ones matmul) with two-stage drains
so the DVE tail never blocks the next matmul stream.
"""

import sys

if "/opt/trn_rl_repo" not in sys.path:
    sys.path.insert(0, "/opt/trn_rl_repo")

import ml_dtypes
import numpy as np

import concourse.mybir as mybir
import concourse.tile as tile
from concourse import bacc, bass_isa, bass_utils

f32 = mybir.dt.float32
f32r = mybir.dt.float32r
bf16 = mybir.dt.bfloat16
AF = mybir.ActivationFunctionType

D = 2048
S = 4096
NH = 16
NKV = 4
HD = 128
SEQW = 4
HEADW = 2
SQ = S // SEQW              # 1024 queries per core
HALO = 1024                 # local window
KVLEN = SQ + HALO           # 2048
QH = NH // HEADW            # 8 q-heads per core
KVH = NKV // HEADW          # 2 kv-heads per core
NB = 512                    # matmul moving block
NSQB = SQ // NB             # 2
NDT = D // 128              # 16
NKT = KVLEN // 128          # 16
EPS = 1.1920929e-07


def build_program():
    nc = bacc.Bacc(
        "TRN2",
        target_bir_lowering=False,
        debug=False,
        enable_asserts=False,
        num_devices=8,
    )
    xtkv = nc.dram_tensor("xtkv", [D, KVLEN], bf16, kind="ExternalInput").ap()
    wqt = nc.dram_tensor("wqt", [D, QH * HD], bf16, kind="ExternalInput").ap()
    wkt = nc.dram_tensor("wkt", [D, KVH * HD], bf16, kind="ExternalInput").ap()
    wvt = nc.dram_tensor("wvt", [D, KVH * HD], bf16, kind="ExternalInput").ap()
    wot = nc.dram_tensor("wot", [QH * HD, D], bf16, kind="ExternalInput").ap()
    ckd = nc.dram_tensor("ck", [128, KVLEN], bf16, kind="ExternalInput").ap()
    skd = nc.dram_tensor("sk", [128, KVLEN], bf16, kind="ExternalInput").ap()
    pzd = nc.dram_tensor("pz", [128, 1], f32, kind="ExternalInput").ap()
    pzcd = nc.dram_tensor("pzc", [128, 8], f32, kind="ExternalInput").ap()
    mskd = nc.dram_tensor(
        "msk", [128, 8, 128], bf16, kind="ExternalInput"
    ).ap()
    outT = nc.dram_tensor("outT", [D, SQ], bf16, kind="ExternalOutput").ap()

    with tile.TileContext(nc) as tc:
        with (
            tc.tile_pool(name="persist", bufs=1) as persist,
            tc.tile_pool(name="scratch", bufs=5) as sc,
            tc.tile_pool(name="rows", bufs=3) as rows,
        ):
            # --- constants ---
            ones_col = persist.tile([128, 1], bf16)
            nc.vector.memset(ones_col, 1.0)
            ones128 = persist.tile([128, 128], bf16)
            nc.vector.memset(ones128, 1.0)
            eps_q = persist.tile([128, 1], f32)
            nc.vector.memset(eps_q, 128.0 * EPS)
            eps_k = persist.tile([128, 1], f32)
            nc.vector.memset(eps_k, EPS)
            pz_sb = persist.tile([128, 1], f32)
            pzc_sb = persist.tile([128, 8], f32)

            ck_sb = persist.tile([128, KVLEN], bf16)
            sk_sb = persist.tile([128, KVLEN], bf16)
            wk_sb = persist.tile([128, NDT, KVH * HD], bf16)
            wv_sb = persist.tile([128, NDT, KVH * HD], bf16)
            msk_sb = persist.tile([128, 8, 128], bf16)
            wot_sb = persist.tile([128, QH, D], bf16)

            qrot = persist.tile([128, QH, SQ], bf16)
            krot = persist.tile([128, KVH, KVLEN], bf16)
            v_sb = persist.tile([128, NKT, KVH * HD], bf16)
            yt = persist.tile([128, QH, SQ], bf16)

            def drain_stage1(acc, s_scale, s_bias, nm, psA, artag="ar"):
                """acc: PSUM [128, NB] raw projection.  Short chain so the PE
                partition-reduce never waits long: raw copy (Act) -> square
                (DVE) -> ones128 matmul (PE) -> sqrt (Act)."""
                raw = sc.tile([128, NB], bf16, tag="big0", name=f"raw{nm}")
                nc.scalar.copy(out=raw, in_=acc)
                sqd_t = sc.tile([128, NB], bf16, tag="big1", name=f"sqd{nm}")
                nc.vector.tensor_mul(out=sqd_t, in0=raw, in1=raw)
                allr = psA.tile([128, NB], f32, tag=artag, name=f"allr{nm}")
                nc.tensor.matmul(
                    allr, lhsT=ones128, rhs=sqd_t, start=True, stop=True
                )
                s_full = sc.tile([128, NB], f32, tag="big3", name=f"sf{nm}")
                nc.scalar.activation(
                    out=s_full, in_=allr, func=AF.Sqrt, bias=s_bias, scale=s_scale
                )
                return raw, s_full

            def drain_stage2(st, out_slice, ctab, stab, nm):
                """Norm+rope tail; runs on DVE/Pool, overlapping the next
                matmul stream."""
                raw, s_full = st
                a_full = sc.tile([128, NB], bf16, tag="big4", name=f"af{nm}")
                with nc.allow_low_precision(reason="f32r is 4-byte fp32 storage"):
                    nc.vector.reciprocal(out=a_full, in_=s_full)
                rawa = sc.tile([128, NB], bf16, tag="big5", name=f"ra{nm}")
                nc.vector.tensor_mul(out=rawa, in0=raw, in1=a_full)
                t1 = sc.tile([128, NB], bf16, tag="big2", name=f"t1{nm}")
                nc.vector.tensor_mul(out=t1, in0=rawa, in1=ctab)
                t2 = sc.tile([128, NB], bf16, tag="big1", name=f"t2{nm}")
                nc.vector.tensor_mul(out=t2, in0=rawa, in1=stab)
                usw = sc.tile([128, NB], bf16, tag="big0", name=f"usw{nm}")
                nc.gpsimd.tensor_copy(out=usw[0:64, :], in_=t2[64:128, :])
                nc.gpsimd.tensor_copy(out=usw[64:128, :], in_=t2[0:64, :])
                nc.vector.tensor_add(
                    out=out_slice[0:64, :], in0=t1[0:64, :], in1=usw[0:64, :]
                )
                nc.vector.tensor_sub(
                    out=out_slice[64:128, :], in0=t1[64:128, :], in1=usw[64:128, :]
                )

            # ====== Projection phase: kv quarters + q blocks interleaved ======
            # Emission order qtr0,1,2, Qblk0, qtr3, Qblk1 so attention's
            # dependencies (krot/v then qrot) drain as early as possible.
            # Per quarter: k-stream (2 accs) then v-stream (4 accs) so k accs
            # complete and drain while v matmuls still run.  Q blocks reuse
            # the quarter's resident x tiles and run heads in 2 groups of 4.
            with tc.tile_pool(name="xs", bufs=6) as xs:
              with tc.tile_pool(name="psP", bufs=6, space="PSUM") as psP, \
                 tc.tile_pool(name="psA", bufs=2, space="PSUM") as psA:
                wk_r = wkt.rearrange("(c p) w -> p c w", p=128)
                wv_r = wvt.rearrange("(c p) w -> p c w", p=128)
                wot_r = wot.rearrange("(y p) d -> p y d", p=128)
                xts = {}

                def load_xtile(qtr, dc):
                    t = xs.tile(
                        [128, 4, NB], bf16, tag="xk", bufs=8, name=f"xk{qtr}_{dc}"
                    )
                    nc.sync.dma_start(
                        out=t,
                        in_=xtkv[
                            512 * dc : 512 * (dc + 1), NB * qtr : NB * (qtr + 1)
                        ].rearrange("(c p) w -> p c w", p=128),
                    )
                    xts[(qtr, dc)] = t

                def emit_qtr(qtr):
                    if qtr == 0:
                        # interleave weight/table loads with the first x tiles
                        for dc in range(4):
                            nc.sync.dma_start(
                                out=wk_sb[:, 4 * dc : 4 * (dc + 1), :],
                                in_=wk_r[:, 4 * dc : 4 * (dc + 1), :],
                            )
                            load_xtile(0, dc)
                        nc.sync.dma_start(out=wv_sb, in_=wv_r)
                        nc.sync.dma_start(out=ck_sb, in_=ckd)
                        nc.sync.dma_start(out=sk_sb, in_=skd)
                        nc.sync.dma_start(out=msk_sb, in_=mskd)
                        nc.sync.dma_start(out=pz_sb, in_=pzd)
                        nc.sync.dma_start(out=pzc_sb, in_=pzcd)
                    else:
                        for dc in range(4):
                            load_xtile(qtr, dc)
                    nc.sync.dma_start(
                        out=wot_sb[:, :, 512 * qtr : 512 * (qtr + 1)],
                        in_=wot_r[:, :, 512 * qtr : 512 * (qtr + 1)],
                    )
                    kacc = {
                        kvh: psP.tile(
                            [128, NB], f32, tag="acc", name=f"kacc{qtr}_{kvh}"
                        )
                        for kvh in range(KVH)
                    }
                    for dc in range(4):
                        for dl in range(4):
                            d = 4 * dc + dl
                            for kvh in range(KVH):
                                nc.tensor.matmul(
                                    kacc[kvh],
                                    lhsT=wk_sb[:, d, HD * kvh : HD * (kvh + 1)],
                                    rhs=xts[(qtr, dc)][:, dl, :],
                                    start=(d == 0),
                                    stop=(d == NDT - 1),
                                )
                    kst = [
                        drain_stage1(
                            kacc[kvh], 1.0 / 128.0, eps_k, f"k{qtr}_{kvh}", psA
                        )
                        for kvh in range(KVH)
                    ]
                    for kvh in range(KVH):
                        drain_stage2(
                            kst[kvh],
                            krot[:, kvh, NB * qtr : NB * (qtr + 1)],
                            ck_sb[:, NB * qtr : NB * (qtr + 1)],
                            sk_sb[:, NB * qtr : NB * (qtr + 1)],
                            f"k{qtr}_{kvh}",
                        )
                    vacc = [
                        psP.tile(
                            [128, KVH * HD], f32, tag="acc", name=f"vacc{qtr}_{lt}"
                        )
                        for lt in range(4)
                    ]
                    for dc in range(4):
                        for dl in range(4):
                            d = 4 * dc + dl
                            for lt in range(4):
                                nc.tensor.matmul(
                                    vacc[lt],
                                    lhsT=xts[(qtr, dc)][:, dl, 128 * lt : 128 * (lt + 1)],
                                    rhs=wv_sb[:, d, :],
                                    start=(d == 0),
                                    stop=(d == NDT - 1),
                                )
                    for lt in range(4):
                        nc.scalar.copy(out=v_sb[:, 4 * qtr + lt, :], in_=vacc[lt])

                def emit_qblk(blk, hgs=(0, 1), gs=4, accpool=None,
                              acctag="acc", arpool=None, artag="ar"):
                    qtr = 2 + blk
                    for hg in hgs:
                        acc = [
                            (accpool or psP).tile(
                                [128, NB], f32, tag=acctag,
                                name=f"qacc{blk}_{hg}_{hh}",
                            )
                            for hh in range(gs)
                        ]
                        for dc in range(4):
                            wq2 = xs.tile(
                                [128, 4, gs * HD], bf16, tag="wqd", bufs=3,
                                name=f"wqd{blk}_{hg}_{dc}",
                            )
                            nc.sync.dma_start(
                                out=wq2,
                                in_=wqt[
                                    512 * dc : 512 * (dc + 1),
                                    gs * HD * hg : gs * HD * (hg + 1),
                                ].rearrange("(c p) w -> p c w", p=128),
                            )
                            for dl in range(4):
                                d = 4 * dc + dl
                                for hh in range(gs):
                                    nc.tensor.matmul(
                                        acc[hh],
                                        lhsT=wq2[:, dl, HD * hh : HD * (hh + 1)],
                                        rhs=xts[(qtr, dc)][:, dl, :],
                                        start=(d == 0),
                                        stop=(d == NDT - 1),
                                    )
                        qst = [
                            drain_stage1(
                                acc[hh], 1.0, eps_q, f"q{blk}_{hg}_{hh}",
                                arpool or psA, artag,
                            )
                            for hh in range(gs)
                        ]
                        for hh in range(gs):
                            h = gs * hg + hh
                            drain_stage2(
                                qst[hh],
                                qrot[:, h, NB * blk : NB * (blk + 1)],
                                ck_sb[:, HALO + NB * blk : HALO + NB * (blk + 1)],
                                sk_sb[:, HALO + NB * blk : HALO + NB * (blk + 1)],
                                f"q{blk}_{hg}_{hh}",
                            )

                emit_qtr(0)
                emit_qtr(1)
                emit_qtr(2)
                emit_qtr(3)
                emit_qblk(0)
                emit_qblk(1)

              # ============ Phase A + O: attention, then out-proj ============
              # 256-query attention blocks: each needs only 10 sk tiles
              # (vs 12 per 512-block) thanks to the sliding window.
              QB = 256
              with tc.tile_pool(name="probs", bufs=6) as pp, tc.tile_pool(
                name="psY", bufs=3, space="PSUM"
            ) as psY, tc.tile_pool(
                name="psR", bufs=1, space="PSUM"
            ) as psR, tc.tile_pool(
                name="psS", bufs=2, space="PSUM"
            ) as psS:
                def attn_block(qb):
                    # 128-query block qb (0..7), both kv heads; 4 q-heads of a
                    # kv group fused into single wide matmuls/exps.  9 sk
                    # tiles: 4 pairs + 1 singleton (the causal edge).
                    qsl = slice(128 * qb, 128 * (qb + 1))
                    for kvh in range(KVH):
                        h0 = 4 * kvh
                        nmg = f"{qb}_{kvh}"
                        yacc4 = psY.tile(
                            [128, 4, 128], f32, tag="y", name=f"y{nmg}"
                        )
                        racc = psR.tile([128, 256], f32, tag="r", name=f"rc{nmg}")
                        rsumA = pp.tile(
                            [128, 2, 128], bf16, tag="rs", bufs=3, name=f"rs{nmg}"
                        )
                        qrh = qrot[:, h0 : h0 + 4, qsl]

                        def scj(j):
                            if j < 4:
                                t = psS.tile(
                                    [128, 8, 128], f32, tag="s", name=f"sa{nmg}_{j}"
                                )
                                for jj in range(2):
                                    kt = qb + 2 * j + jj
                                    nc.tensor.matmul(
                                        t[:, 4 * jj : 4 * jj + 4, :],
                                        lhsT=krot[
                                            :, kvh, 128 * kt : 128 * (kt + 1)
                                        ],
                                        rhs=qrh,
                                        start=True,
                                        stop=True,
                                    )
                            else:
                                t = psS.tile(
                                    [128, 4, 128], f32, tag="s", name=f"sa{nmg}_4"
                                )
                                kt = qb + 8
                                nc.tensor.matmul(
                                    t,
                                    lhsT=krot[:, kvh, 128 * kt : 128 * (kt + 1)],
                                    rhs=qrh,
                                    start=True,
                                    stop=True,
                                )
                            return t

                        def em(j, t):
                            if j < 4:
                                pt = pp.tile(
                                    [128, 8, 128], bf16, tag="pt", bufs=6,
                                    name=f"pt{nmg}_{j}",
                                )
                            else:
                                pt = pp.tile(
                                    [128, 4, 128], bf16, tag="pt", bufs=6,
                                    name=f"pt{nmg}_4",
                                )
                            nc.scalar.activation(
                                out=pt, in_=t, func=AF.Exp, bias=0.0, scale=1.0
                            )
                            if j == 0:
                                nc.vector.tensor_mul(
                                    out=pt[:, 0:4, :],
                                    in0=pt[:, 0:4, :],
                                    in1=msk_sb[:, 0:4, :],
                                )
                            elif j == 4:
                                nc.vector.tensor_mul(
                                    out=pt, in0=pt, in1=msk_sb[:, 4:8, :]
                                )
                            return pt

                        def pv(j, pt):
                            if j < 4:
                                for jj in range(2):
                                    tt = 2 * j + jj
                                    kt = qb + tt
                                    nc.tensor.matmul(
                                        yacc4,
                                        lhsT=v_sb[:, kt, HD * kvh : HD * (kvh + 1)],
                                        rhs=pt[:, 4 * jj : 4 * jj + 4, :],
                                        start=(tt == 0),
                                        stop=False,
                                    )
                                    nc.tensor.matmul(
                                        racc,
                                        lhsT=ones128,
                                        rhs=pt[:, 4 * jj + 2 : 4 * jj + 4, :],
                                        start=(tt == 0),
                                        stop=False,
                                    )
                                if j == 0:
                                    nc.vector.tensor_add(
                                        out=rsumA,
                                        in0=pt[:, 0:2, :],
                                        in1=pt[:, 4:6, :],
                                    )
                                else:
                                    tmp = pp.tile(
                                        [128, 2, 128], bf16, tag="rt", bufs=3,
                                        name=f"rt{nmg}_{j}",
                                    )
                                    nc.vector.tensor_add(
                                        out=tmp,
                                        in0=pt[:, 0:2, :],
                                        in1=pt[:, 4:6, :],
                                    )
                                    nc.vector.tensor_add(
                                        out=rsumA, in0=rsumA, in1=tmp
                                    )
                            else:
                                kt = qb + 8
                                nc.tensor.matmul(
                                    yacc4,
                                    lhsT=v_sb[:, kt, HD * kvh : HD * (kvh + 1)],
                                    rhs=pt,
                                    start=False,
                                    stop=True,
                                )
                                nc.tensor.matmul(
                                    racc,
                                    lhsT=ones128,
                                    rhs=pt[:, 2:4, :],
                                    start=False,
                                    stop=True,
                                )
                                nc.vector.tensor_add(
                                    out=rsumA, in0=rsumA, in1=pt[:, 0:2, :]
                                )

                        # depth-2 pipeline over 4 pairs + singleton
                        sacc, ptb = {}, {}
                        sacc[0] = scj(0)
                        sacc[1] = scj(1)
                        ptb[0] = em(0, sacc.pop(0))
                        for j in range(5):
                            if j + 2 <= 4:
                                sacc[j + 2] = scj(j + 2)
                            if j + 1 <= 4:
                                ptb[j + 1] = em(j + 1, sacc.pop(j + 1))
                            pv(j, ptb.pop(j))

                        # heads h0+2/h0+3 via the PE racc; then its psR slot
                        # frees for heads h0/h0+1's partition-reduce
                        nc.vector.tensor_scalar_sub(
                            out=racc, in0=racc,
                            scalar1=pzc_sb[:, qb : qb + 1],
                        )
                        rbB = sc.tile([128, 256], f32r, tag="big5", name=f"rbB{nmg}")
                        with nc.allow_low_precision(reason="f32r 4-byte"):
                            nc.vector.reciprocal(out=rbB, in_=racc)
                        for i in range(2):
                            nc.vector.tensor_mul(
                                out=yt[:, h0 + 2 + i, qsl],
                                in0=yacc4[:, 2 + i, :],
                                in1=rbB[:, 128 * i : 128 * (i + 1)],
                            )
                        rallA = psR.tile([128, 256], f32, tag="r", name=f"rlA{nmg}")
                        nc.tensor.matmul(
                            rallA, lhsT=ones128, rhs=rsumA, start=True, stop=True
                        )
                        nc.vector.tensor_scalar_sub(
                            out=rallA, in0=rallA,
                            scalar1=pzc_sb[:, qb : qb + 1],
                        )
                        rinvA = sc.tile(
                            [128, 256], f32r, tag="big5", name=f"rbA{nmg}"
                        )
                        with nc.allow_low_precision(reason="f32r 4-byte"):
                            nc.vector.reciprocal(out=rinvA, in_=rallA)
                        for i in range(2):
                            nc.vector.tensor_mul(
                                out=yt[:, h0 + i, qsl],
                                in0=yacc4[:, i, :],
                                in1=rinvA[:, 128 * i : 128 * (i + 1)],
                            )

                def outproj(bo):
                  # ---- out-proj for this 512-block (weights resident) ----
                  for dm in range(NDT):
                        oacc = psS.tile(
                            [128, NB], f32, tag="s", name=f"oacc{dm}_{bo}"
                        )
                        for y in range(QH):
                            nc.tensor.matmul(
                                oacc,
                                lhsT=wot_sb[:, y, 128 * dm : 128 * (dm + 1)],
                                rhs=yt[:, y, NB * bo : NB * (bo + 1)],
                                start=(y == 0),
                                stop=(y == QH - 1),
                            )
                        ot = sc.tile([128, NB], bf16, tag="big0", name=f"ot{dm}_{bo}")
                        if bo == NSQB - 1 and dm == NDT - 1:
                            for hv in range(2):
                                hs = slice(256 * hv, 256 * (hv + 1))
                                nc.vector.tensor_copy(
                                    out=ot[:, hs], in_=oacc[:, hs]
                                )
                                nc.sync.dma_start(
                                    out=outT[
                                        128 * dm : 128 * (dm + 1),
                                        NB * bo + 256 * hv : NB * bo + 256 * (hv + 1),
                                    ],
                                    in_=ot[:, hs],
                                )
                        else:
                            nc.vector.tensor_copy(out=ot, in_=oacc)
                            nc.sync.dma_start(
                                out=outT[
                                    128 * dm : 128 * (dm + 1),
                                    NB * bo : NB * (bo + 1),
                                ],
                                in_=ot,
                            )

                for qb in range(4):
                    attn_block(qb)
                outproj(0)
                for qb in range(4, 8):
                    attn_block(qb)
                outproj(1)

    nc.compile()
    return nc


def host_prep(x, Wq, Wk, Wv, Wo):
    x2 = np.asarray(x, dtype=np.float32).reshape(S, D)
    xT = np.ascontiguousarray(x2.T)
    WqT = np.ascontiguousarray(np.asarray(Wq, np.float32).T)
    WkT = np.ascontiguousarray(np.asarray(Wk, np.float32).T)
    WvT = np.ascontiguousarray(np.asarray(Wv, np.float32).T)
    WoT = np.ascontiguousarray(np.asarray(Wo, np.float32).T)

    pos = np.arange(-HALO, S, dtype=np.float32)
    invf = 1.0 / (10000.0 ** (np.arange(0, HD, 2, dtype=np.float32) / HD))
    fr = pos[:, None] * invf[None, :]
    cosT = np.cos(fr).T.astype(np.float32)
    sinT = np.sin(fr).T.astype(np.float32)
    C2 = np.ascontiguousarray(np.concatenate([cosT, cosT], axis=0))
    S2 = np.ascontiguousarray(np.concatenate([sinT, sinT], axis=0))

    in_maps = []
    for si in range(SEQW):
        lo = si * SQ - HALO
        xtkv = np.zeros((D, KVLEN), np.float32)
        lo_c = max(lo, 0)
        xtkv[:, lo_c - lo :] = xT[:, lo_c : si * SQ + SQ]
        cks = C2[:, HALO + lo : HALO + lo + KVLEN].astype(ml_dtypes.bfloat16)
        sks = S2[:, HALO + lo : HALO + lo + KVLEN].astype(ml_dtypes.bfloat16)
        pz = np.full((128, 1), 0.0 if si == 0 else 1.0, np.float32)

        # 0/1 prob masks for the window/causal edge sk-tile pairs, with the
        # first-shard halo baked in.  msk[p, 4*blk+im, jj*NB+c] corresponds to
        # key 128*(4*blk+2*ip+jj)+p and query 512*blk+c of this shard.
        # edge-tile prob masks for 128-query blocks in the 4-head-fused
        # layout: slots 0-3 window tile t=0 (x4 heads), 4-7 causal tile t=8.
        # Patterns are block-independent; first-shard halo baked in.  Interior
        # halo tiles on the first shard are NOT masked: their probs are
        # exp(0)=1 with v=0, so only the softmax denominator needs fixing,
        # via the per-block halo count pzc.
        msk = np.zeros((128, 8, 128), np.float32)
        p_i = np.arange(128)
        c_i = np.arange(128)
        for im, t in enumerate((0, 0, 0, 0, 8, 8, 8, 8)):
            k_abs = si * SQ - HALO + 128 * t + p_i[:, None]
            q_abs = si * SQ + c_i[None, :]
            msk[:, im, :] = (
                (k_abs <= q_abs) & (k_abs > q_abs - HALO) & (k_abs >= 0)
            )
        msk_b = msk.astype(ml_dtypes.bfloat16)
        pzc = np.zeros((128, 8), np.float32)
        if si == 0:
            for qb in range(8):
                pzc[:, qb] = 128.0 * max(0, 7 - qb)

        xtkv_b = xtkv.astype(ml_dtypes.bfloat16)
        for hi in range(HEADW):
            in_maps.append(
                dict(
                    xtkv=xtkv_b,
                    wqt=WqT[:, 1024 * hi : 1024 * (hi + 1)].astype(ml_dtypes.bfloat16),
                    wkt=WkT[:, 256 * hi : 256 * (hi + 1)].astype(ml_dtypes.bfloat16),
                    wvt=WvT[:, 256 * hi : 256 * (hi + 1)].astype(ml_dtypes.bfloat16),
                    wot=WoT[1024 * hi : 1024 * (hi + 1), :].astype(ml_dtypes.bfloat16),
                    ck=cks,
                    sk=sks,
                    pz=pz,
                    pzc=pzc,
                    msk=msk_b,
                )
            )
    return in_maps


def host_post(results):
    out = np.empty((S, D), np.float32)
    for si in range(SEQW):
        acc = results[2 * si]["outT"].astype(np.float32) + results[
            2 * si + 1
        ]["outT"].astype(np.float32)
        out[si * SQ : (si + 1) * SQ, :] = acc.T
    return out.reshape(1, S, D)


_cached_nc = None


def get_nc():
    global _cached_nc
    if _cached_nc is None:
        _cached_nc = build_program()
    return _cached_nc


def kernel(**inputs):
    nc = get_nc()
    in_maps = host_prep(
        inputs["x"], inputs["Wq"], inputs["Wk"], inputs["Wv"], inputs["Wo"]
    )
    res = bass_utils.run_bass_kernel_spmd(nc, in_maps, core_ids=list(range(8)))
    return host_post(res.results)
